# revision 6
# baseline (speedup 1.0000x reference)
"""Trainium2 Bass kernel for the GroupNorm + single-head spatial attention block.

Reference computation (per batch b):
    n  = GroupNorm(x, groups=4) * gn_w + gn_b          x: [C=256, N=1024]
    Q  = Wq @ n + bq ; K = Wk @ n + bk ; V = Wv @ n + bv
    S  = Q^T K / sqrt(C)                                [N, N]
    A  = softmax(S, axis=-1)
    U  = V @ A^T                                        [C, N]
    y  = x + Wo @ U + bo

Strategy (data-parallel over batch, 2 batches per NeuronCore, 8 cores):
  - everything stays in fp32 in SBUF; matmuls run as float32r (full PE rate
    for free-dim >= 256).
  - S is computed TRANSPOSED (S^T = K_tile^T @ Q, j on partitions) so that
    the exp result E^T = exp(S^T/16) is directly usable as the moving operand
    of U = V @ E^T (contraction over j on partitions).  No [N,N] transpose.
  - softmax skips the max-subtraction (scores are O(1): |S|/16 < ~10, exp is
    safe in fp32); the denominator is sum over j (= partitions), computed by
    accumulating E^T tiles on DVE and a final ones^T matmul; the reciprocal
    is broadcast back to 128 partitions with a K=1 ones matmul and applied
    to U (per-output-column scaling commutes with the V contraction).
  - GroupNorm stats: bn_stats/bn_aggr per partition, then a [128,2]
    indicator matmul reduces (mean, E[x^2]) over each group's 64 partitions,
    and a [2,128] indicator matmul broadcasts (mean, rstd) back.
"""

import os
import numpy as np

import concourse.bass as bass
import concourse.bacc as bacc
import concourse.tile as tile
import concourse.bass_utils as bass_utils
from concourse import mybir
from concourse.alu_op_type import AluOpType

P = 128
B, C, H, W = 16, 256, 32, 32
N = H * W                 # 1024
N_CORES = 8
BPC = B // N_CORES        # batches per core
CT = C // P               # 2 c-tiles
JT = N // P               # 8 j-tiles
FH = 512                  # free-dim half (max fp32 moving operand)
IH = N // FH              # 2 i-halves
GROUPS = 4
GSIZE = C // GROUPS       # 64 channels per group
EPS = 1e-5
SCALE = 1.0 / float(np.sqrt(C))

F32 = mybir.dt.float32
F32R = mybir.dt.float32r

AF = mybir.ActivationFunctionType

USE_F32R = os.environ.get("ATTN_F32R", "1") == "1"
MM_DT = F32R if USE_F32R else F32


def _mm(ap):
    """Matmul-operand view: float32r runs at full PE rate (>=256 free dim)."""
    if not USE_F32R or ap.dtype == F32R:
        return ap
    return ap.bitcast(F32R)


def _rnd(ap):
    """Producer-side view: outputs consumed by f32r matmuls must be written
    with an f32r output dtype so the engine rounds them (BIR verifier rule)."""
    if not USE_F32R or ap.dtype == F32R:
        return ap
    return ap.bitcast(F32R)


def _build_batch(nc, tc, pools, aps, b):
    """Emit instructions for one batch b (0..BPC-1)."""
    (consts, xpool, npool, qkpool, vtpool, etpool, accpool, rcpool, upool,
     ypool, small, p_big, p_u) = pools

    sl = [slice(ih * FH, (ih + 1) * FH) for ih in range(IH)]

    # ---- load x ----
    x_sb = xpool.tile([P, CT, N], F32, tag="x")
    nc.sync.dma_start(out=x_sb[:], in_=aps["x"][b])

    # ---- GroupNorm ----
    n_sb = npool.tile([P, CT, N], F32, tag="n")
    for t in range(CT):
        xt = x_sb[:, t, :]
        resh = xt.rearrange("p (s f) -> p s f", f=FH)
        stats6 = small.tile([P, IH, 6], F32, tag="stats6")
        for s in range(IH):
            nc.vector.bn_stats(out=stats6[:, s, :], in_=resh[:, s, :])
        mv = small.tile([P, 2], F32, tag="mv")
        nc.vector.bn_aggr(out=mv[:], in_=stats6[:])
        # pq = (mean, E[x^2]) per partition
        pq = small.tile([P, 2], F32, tag="pq")
        nc.vector.tensor_copy(pq[:, 0:1], mv[:, 0:1])
        nc.vector.tensor_mul(pq[:, 1:2], mv[:, 0:1], mv[:, 0:1])
        nc.vector.tensor_add(pq[:, 1:2], pq[:, 1:2], mv[:, 1:2])
        # group-reduce over partitions: [2,2] = (mean_g, E[x^2]_g)
        stats_ps = p_big.tile([2, 2], F32, tag="big")
        nc.tensor.matmul(stats_ps[:], aps["ind_fwd"][:], pq[:],
                         start=True, stop=True)
        s_sb = small.tile([2, 2], F32, tag="s2")
        nc.vector.tensor_copy(s_sb[:], stats_ps[:])
        msq = small.tile([2, 1], F32, tag="msq")
        nc.vector.tensor_mul(msq[:], s_sb[:, 0:1], s_sb[:, 0:1])
        nc.vector.tensor_sub(msq[:], s_sb[:, 1:2], msq[:])      # var
        nc.scalar.activation(out=msq[:], in_=msq[:], func=AF.Sqrt,
                             bias=aps["eps"][:])                # sqrt(var+eps)
        nc.vector.reciprocal(out=s_sb[:, 1:2], in_=msq[:])      # rstd
        # broadcast (mean, rstd) to the 128 partitions
        bc_ps = p_big.tile([P, 2], F32, tag="big")
        nc.tensor.matmul(bc_ps[:], aps["ind_bwd"][:], s_sb[:],
                         start=True, stop=True)
        # fold gamma/beta: n = x * (rstd*w) + (b - mean*rstd*w)
        sc = small.tile([P, 2], F32, tag="sc")
        nc.vector.tensor_mul(sc[:, 0:1], bc_ps[:, 1:2], aps["gnw"][:, t:t + 1])
        nc.vector.tensor_mul(sc[:, 1:2], bc_ps[:, 0:1], sc[:, 0:1])
        nc.vector.tensor_sub(sc[:, 1:2], aps["gnb"][:, t:t + 1], sc[:, 1:2])
        nc.vector.tensor_scalar(out=_rnd(n_sb[:, t, :]), in0=xt,
                                scalar1=sc[:, 0:1], scalar2=sc[:, 1:2],
                                op0=AluOpType.mult, op1=AluOpType.add)

    # ---- Q, K projections: [C, N] (c on partitions) ----
    q_sb = qkpool.tile([P, CT, N], F32, tag="q")
    k_sb = qkpool.tile([P, CT, N], F32, tag="k")
    for (w_sb, b_sb, dst) in ((aps["wq"], aps["bq"], q_sb),
                              (aps["wk"], aps["bk"], k_sb)):
        for ot in range(CT):
            pr_ps = p_big.tile([P, N], F32, tag="big")
            for ih in range(IH):
                for kt in range(CT):
                    nc.tensor.matmul(
                        pr_ps[:, sl[ih]],
                        _mm(w_sb[:, kt, ot * P:(ot + 1) * P]),
                        _mm(n_sb[:, kt, sl[ih]]),
                        start=(kt == 0), stop=(kt == CT - 1))
            nc.scalar.activation(out=_rnd(dst[:, ot, :]), in_=pr_ps[:],
                                 func=AF.Identity, bias=b_sb[:, ot:ot + 1])

    # ---- V^T: [N, C] (j on partitions), computed directly as n^T @ Wv^T ----
    vt_sb = vtpool.tile([P, JT, C], F32, tag="vt")
    for jt in range(JT):
        vt_ps = p_big.tile([P, C], F32, tag="big")
        for kt in range(CT):
            nc.tensor.matmul(vt_ps[:],
                             _mm(n_sb[:, kt, jt * P:(jt + 1) * P]),
                             _mm(aps["wv"][:, kt, :]),
                             start=(kt == 0), stop=(kt == CT - 1))
        nc.vector.tensor_add(_rnd(vt_sb[:, jt, :]), vt_ps[:], aps["bv_bc"][:])

    # ---- attention: S^T -> exp -> (colsum, U-accumulate) per j-tile ----
    u_ps = [p_u.tile([P, FH], F32, tag="u", name=f"u_ps{b}_{i}")
            for i in range(CT * IH)]
    acc_sb = accpool.tile([P, N], F32, tag="acc")
    for jt in range(JT):
        st_ps = p_big.tile([P, N], F32, tag="big")
        for ih in range(IH):
            for kt in range(CT):
                nc.tensor.matmul(
                    st_ps[:, sl[ih]],
                    _mm(k_sb[:, kt, jt * P:(jt + 1) * P]),
                    _mm(q_sb[:, kt, sl[ih]]),
                    start=(kt == 0), stop=(kt == CT - 1))
        et = etpool.tile([P, N], F32, tag="et")
        for ih in range(IH):
            nc.scalar.activation(out=_rnd(et[:, sl[ih]]), in_=st_ps[:, sl[ih]],
                                 func=AF.Exp, scale=SCALE)
        if jt == 0:
            nc.vector.tensor_copy(_rnd(acc_sb[:]), et[:])
        else:
            nc.vector.tensor_add(_rnd(acc_sb[:]), acc_sb[:], et[:])
        for ci in range(CT):
            for ih in range(IH):
                nc.tensor.matmul(
                    u_ps[ci * IH + ih][:],
                    _mm(vt_sb[:, jt, ci * P:(ci + 1) * P]),
                    _mm(et[:, sl[ih]]),
                    start=(jt == 0), stop=(jt == JT - 1))

    # ---- softmax denominator: reduce acc over partitions, recip, broadcast
    csum_sb = rcpool.tile([1, N], F32, tag="csum")
    rc_sb = rcpool.tile([P, N], F32, tag="rc")
    for ih in range(IH):
        csum_ps = p_big.tile([1, FH], F32, tag="big")
        nc.tensor.matmul(csum_ps[:], _mm(aps["ones_col"][:]),
                         _mm(acc_sb[:, sl[ih]]), start=True, stop=True)
        nc.scalar.activation(out=_rnd(csum_sb[:, sl[ih]]), in_=csum_ps[:],
                             func=AF.Copy)
    for ih in range(IH):
        bc_ps = p_big.tile([P, FH], F32, tag="big")
        nc.tensor.matmul(bc_ps[:], _mm(aps["ones_row"][:]),
                         _mm(csum_sb[:, sl[ih]]), start=True, stop=True)
        nc.vector.reciprocal(out=rc_sb[:, sl[ih]], in_=bc_ps[:])

    # ---- normalize U ----
    u_sb = upool.tile([P, CT, N], F32, tag="u_sb")
    for ci in range(CT):
        for ih in range(IH):
            nc.vector.tensor_mul(_rnd(u_sb[:, ci, sl[ih]]), u_ps[ci * IH + ih][:],
                                 rc_sb[:, sl[ih]])

    # ---- output projection + bias + residual ----
    y_sb = ypool.tile([P, CT, N], F32, tag="y")
    for ot in range(CT):
        o_ps = p_big.tile([P, N], F32, tag="big")
        for ih in range(IH):
            for ci in range(CT):
                nc.tensor.matmul(
                    o_ps[:, sl[ih]],
                    _mm(aps["wo"][:, ci, ot * P:(ot + 1) * P]),
                    _mm(u_sb[:, ci, sl[ih]]),
                    start=(ci == 0), stop=(ci == CT - 1))
        for ih in range(IH):
            nc.vector.scalar_tensor_tensor(
                out=y_sb[:, ot, sl[ih]], in0=o_ps[:, sl[ih]],
                scalar=aps["bo"][:, ot:ot + 1], in1=x_sb[:, ot, sl[ih]],
                op0=AluOpType.add, op1=AluOpType.add)

    nc.sync.dma_start(out=aps["y"][b], in_=y_sb[:])


def _build():
    nc = bacc.Bacc("TRN2", target_bir_lowering=False, debug=False,
                   num_devices=N_CORES)

    x_d = nc.dram_tensor("x", [BPC, C, N], F32, kind="ExternalInput")
    y_d = nc.dram_tensor("y", [BPC, C, N], F32, kind="ExternalOutput")
    w_d = {k: nc.dram_tensor(k, [C, C], MM_DT, kind="ExternalInput")
           for k in ("wqT", "wkT", "wvT", "woT")}
    b_d = {k: nc.dram_tensor(k, [C], F32, kind="ExternalInput")
           for k in ("bq", "bk", "bv", "bo", "gn_w", "gn_b")}
    ind_fwd_d = nc.dram_tensor("ind_fwd", [P, 2], F32, kind="ExternalInput")
    ind_bwd_d = nc.dram_tensor("ind_bwd", [2, P], F32, kind="ExternalInput")
    ones_col_d = nc.dram_tensor("ones_col", [P, 1], MM_DT, kind="ExternalInput")
    ones_row_d = nc.dram_tensor("ones_row", [1, P], MM_DT, kind="ExternalInput")

    with tile.TileContext(nc) as tc:
        with (
            tc.tile_pool(name="consts", bufs=1) as consts,
            tc.tile_pool(name="xpool", bufs=2) as xpool,
            tc.tile_pool(name="npool", bufs=2) as npool,
            tc.tile_pool(name="qkpool", bufs=2) as qkpool,
            tc.tile_pool(name="vtpool", bufs=2) as vtpool,
            tc.tile_pool(name="etpool", bufs=3) as etpool,
            tc.tile_pool(name="accpool", bufs=2) as accpool,
            tc.tile_pool(name="rcpool", bufs=2) as rcpool,
            tc.tile_pool(name="upool", bufs=2) as upool,
            tc.tile_pool(name="ypool", bufs=2) as ypool,
            tc.tile_pool(name="small", bufs=4) as small,
            tc.tile_pool(name="p_big", bufs=2, space="PSUM") as p_big,
            tc.tile_pool(name="p_u", bufs=CT * IH, space="PSUM") as p_u,
        ):
            aps = {}
            # weights, transposed on host: w[k][c', o] with c' contraction
            for k, dst in (("wqT", "wq"), ("wkT", "wk"),
                           ("wvT", "wv"), ("woT", "wo")):
                t_ = consts.tile([P, CT, C], MM_DT, tag=dst)
                nc.sync.dma_start(
                    out=t_[:],
                    in_=w_d[k].ap().rearrange("(t p) o -> p t o", p=P))
                aps[dst] = t_
            for k, dst in (("bq", "bq"), ("bk", "bk"), ("bo", "bo"),
                           ("gn_w", "gnw"), ("gn_b", "gnb")):
                t_ = consts.tile([P, CT], F32, tag=dst)
                nc.sync.dma_start(
                    out=t_[:], in_=b_d[k].ap().rearrange("(t p) -> p t", p=P))
                aps[dst] = t_
            bv_bc = consts.tile([P, C], F32, tag="bv_bc")
            bv_ap = b_d["bv"].ap()
            nc.sync.dma_start(
                out=bv_bc[:],
                in_=bass.AP(tensor=bv_ap.tensor, offset=bv_ap.offset,
                            ap=[[0, P]] + list(bv_ap.ap)))
            aps["bv_bc"] = bv_bc
            for k, d_, shape in (("ind_fwd", ind_fwd_d, [P, 2]),
                                 ("ind_bwd", ind_bwd_d, [2, P]),
                                 ("ones_col", ones_col_d, [P, 1]),
                                 ("ones_row", ones_row_d, [1, P])):
                dt_ = MM_DT if k in ("ones_col", "ones_row") else F32
                t_ = consts.tile(shape, dt_, tag=k)
                nc.sync.dma_start(out=t_[:], in_=d_.ap())
                aps[k] = t_
            eps_t = consts.tile([2, 1], F32, tag="eps")
            nc.vector.memset(eps_t[:], EPS)
            aps["eps"] = eps_t

            aps["x"] = x_d.ap().rearrange("b (t p) n -> b p t n", p=P)
            aps["y"] = y_d.ap().rearrange("b (t p) n -> b p t n", p=P)

            pools = (consts, xpool, npool, qkpool, vtpool, etpool, accpool,
                     rcpool, upool, ypool, small, p_big, p_u)
            for b in range(BPC):
                _build_batch(nc, tc, pools, aps, b)

    nc.compile()
    return nc


_NC = None


def _get_nc():
    global _NC
    if _NC is None:
        _NC = _build()
    return _NC


def _make_in_maps(inputs):
    f32 = lambda a: np.ascontiguousarray(np.asarray(a, dtype=np.float32))
    x = f32(inputs["x"]).reshape(B, C, N)
    shared = {
        "wqT": f32(np.asarray(inputs["Wq"], dtype=np.float32).T),
        "wkT": f32(np.asarray(inputs["Wk"], dtype=np.float32).T),
        "wvT": f32(np.asarray(inputs["Wv"], dtype=np.float32).T),
        "woT": f32(np.asarray(inputs["Wo"], dtype=np.float32).T),
        "bq": f32(inputs["bq"]), "bk": f32(inputs["bk"]),
        "bv": f32(inputs["bv"]), "bo": f32(inputs["bo"]),
        "gn_w": f32(inputs["gn_w"]), "gn_b": f32(inputs["gn_b"]),
    }
    ind_fwd = np.zeros((P, 2), np.float32)
    ind_fwd[:GSIZE, 0] = 1.0 / (GSIZE * 1.0)
    ind_fwd[GSIZE:, 1] = 1.0 / (GSIZE * 1.0)
    ind_bwd = np.zeros((2, P), np.float32)
    ind_bwd[0, :GSIZE] = 1.0
    ind_bwd[1, GSIZE:] = 1.0
    shared["ind_fwd"] = ind_fwd
    shared["ind_bwd"] = ind_bwd
    shared["ones_col"] = np.ones((P, 1), np.float32)
    shared["ones_row"] = np.ones((1, P), np.float32)

    in_maps = []
    for m in range(N_CORES):
        im = dict(shared)
        im["x"] = np.ascontiguousarray(x[m * BPC:(m + 1) * BPC])
        in_maps.append(im)
    return in_maps


def _gather(results):
    y = np.concatenate([r["y"] for r in results], axis=0)
    return np.ascontiguousarray(y.reshape(B, C, H, W).astype(np.float32))


def kernel(**inputs):
    nc = _get_nc()
    res = bass_utils.run_bass_kernel_spmd(nc, _make_in_maps(inputs),
                                          core_ids=list(range(N_CORES)))
    return _gather(res.results)


def _ensure_ntff_hook():
    """The agent image lacks antenv.axon_hooks; synthesize it and install the
    ctypes-based NTFF hook from trn_agent_boot so trace=True works locally."""
    import sys
    import types
    try:
        from antenv.axon_hooks import get_axon_ntff_profile_hook  # noqa: F401
        return
    except ImportError:
        pass
    hook = None
    try:
        from trn_agent_boot.trn_boot import _ntff_profile_via_ctypes
        hook = _ntff_profile_via_ctypes("/opt/axon/libaxon_pjrt.so")
    except Exception:
        hook = None
    mod = types.ModuleType("antenv.axon_hooks")
    mod.get_axon_ntff_profile_hook = lambda: hook
    mod.set_axon_ntff_profile_hook = lambda h: None
    sys.modules["antenv.axon_hooks"] = mod
    # keep artifacts local: no bucket in this sandbox
    bass_utils.upload_artifacts = lambda d: d


def kernel_traced(**inputs):
    """Returns (output, exec_time_ns, trace_path) using NTFF profiling."""
    _ensure_ntff_hook()
    nc = _get_nc()
    res = bass_utils.run_bass_kernel_spmd(nc, _make_in_maps(inputs),
                                          core_ids=list(range(N_CORES)),
                                          trace=True)
    trace_path = None
    if res.instructions_and_trace is not None:
        trace_path = res.instructions_and_trace[1]
    return _gather(res.results), res.exec_time_ns, trace_path
